# revision 4
# baseline (speedup 1.0000x reference)
"""2-layer dense GCN on 8 Trainium2 NeuronCores.

Reference computation (all fp32):
    H0 = relu((A_norm @ X) @ W0)
    H1 = relu((A_norm @ H0) @ W1)
A_norm: [16384, 16384], X: [16384, 128], W0/W1: [128, 128].

Sharding: 1D row partition of A_norm (2048 rows/core). Each core holds
A[rows_c].T (host-transposed so the node-contraction dim lands on SBUF
partitions), computes its row block of each layer, and the hidden state
is exchanged between layers with an on-device AllGather.

Device layout is transpose-free:
  - aggregate:  psum[d, i] += X_tile[j, d].T @ A_T_tile[j, i]
                (lhsT = stationary node-major X/H tile, rhs = A^T slice)
  - linear:     psum[i, e]  = M^T_tile[d, i].T @ W[d, e]   (node-major out)
  - relu fused into the PSUM->SBUF eviction on the scalar engine.

The stationary H layout in SBUF ([128, rows] chunks, one per rank) is
exactly what AllGather produces, so no transposes are needed anywhere;
the only data reshuffle is the host-side A[rows].T at sharding time.

PRECISION modes:
  - "fp32":   exact fp32 matmuls (4 cyc/row on the PE).
  - "split3": A and X/H split into bf16 hi+lo; aggregate computed as
              Ah@Xh + Ah@Xl + Al@Xh (3 bf16 passes, 3 cyc/row,
              ~6e-6 relative error — fp32-class).
  - "bf16":   plain bf16 aggregate (1 cyc/row, half the DMA bytes,
              ~3e-3 relative error).
"""

import sys
from contextlib import ExitStack

if "/opt/trn_rl_repo" not in sys.path:
    sys.path.insert(0, "/opt/trn_rl_repo")

import numpy as np

N_NODES = 16384
D = 128
NCORES = 8
ROWS = N_NODES // NCORES  # 2048

PRECISION = "fp32"  # "fp32" | "split3" | "bf16"


def build_gcn(n_nodes=N_NODES, d=D, ncores=NCORES, precision=PRECISION):
    """Build the SPMD Bass program (one program, runs on all cores)."""
    import concourse.bass as bass  # noqa: F401
    import concourse.tile as tile
    from concourse import bacc, mybir

    F32 = mybir.dt.float32
    BF16 = mybir.dt.bfloat16
    agg_dt = F32 if precision == "fp32" else BF16
    esz = 4 if precision == "fp32" else 2
    nsplit = 2 if precision == "split3" else 1  # hi/lo operand copies

    rows = n_nodes // ncores
    jt = n_nodes // 128          # total j-tiles (contraction tiles)
    jt_per_rank = jt // ncores   # j-tiles covered by one stat chunk
    ic = min(512, rows)          # i-chunk width (one PSUM bank, fp32 out)
    nch = rows // ic             # i-chunks per core
    lt = ic // 128               # linear i-tiles per chunk
    # j-tiles per A DMA: target ~2 MiB per transfer
    jg = max(1, (2 * 1024 * 1024) // (128 * rows * esz))
    jg = min(jg, jt_per_rank)
    ndma = jt // jg

    nc = bacc.Bacc("TRN2", target_bir_lowering=False, num_devices=ncores)

    # A^T shards (hi always; lo only for split3)
    a_in = [
        nc.dram_tensor(f"a{s}", [n_nodes, rows], agg_dt, kind="ExternalInput")
        for s in range(nsplit)
    ]
    # x_t: X pre-tiled on host into the same layout AllGather produces:
    # x_t[r*128 + p, tl*128 + dd] = X[(r*jt_per_rank + tl)*128 + p, dd]
    x_in = [
        nc.dram_tensor(f"x{s}", [ncores * 128, rows], agg_dt, kind="ExternalInput")
        for s in range(nsplit)
    ]
    w0 = nc.dram_tensor("w0", [d, d], F32, kind="ExternalInput")
    w1 = nc.dram_tensor("w1", [d, d], F32, kind="ExternalInput")
    h_out = nc.dram_tensor("h_out", [rows, d], F32, kind="ExternalOutput")

    relu = mybir.ActivationFunctionType.Relu

    with tile.TileContext(nc) as tc, ExitStack() as ctx:
        sb1 = ctx.enter_context(tc.tile_pool(name="sb1", bufs=1))
        stat_pool = ctx.enter_context(
            tc.tile_pool(name="stat", bufs=ncores * nsplit)
        )
        a_pool = ctx.enter_context(tc.tile_pool(name="a", bufs=4 * nsplit))
        m_pool = ctx.enter_context(tc.tile_pool(name="m", bufs=2))
        h_pool = ctx.enter_context(tc.tile_pool(name="h", bufs=4))
        split_pool = ctx.enter_context(tc.tile_pool(name="spl", bufs=4))
        agg_pool = ctx.enter_context(tc.tile_pool(name="agg", bufs=nch, space="PSUM"))
        lin_pool = ctx.enter_context(tc.tile_pool(name="lin", bufs=2, space="PSUM"))
        dram = ctx.enter_context(tc.tile_pool(name="dram", bufs=1, space="DRAM"))

        w0_sb = sb1.tile([d, d], F32)
        nc.sync.dma_start(out=w0_sb[:], in_=w0[:])
        w1_sb = sb1.tile([d, d], F32)
        nc.sync.dma_start(out=w1_sb[:], in_=w1[:])

        def load_stat_chunks(srcs):
            """srcs: list of [ncores*128, rows] DRAM views -> per-split chunk
            tile lists."""
            out = []
            for s, src in enumerate(srcs):
                chunks = []
                for r in range(ncores):
                    sc = stat_pool.tile(
                        [128, rows], agg_dt, name=f"sc{s}_{r}", tag="sc"
                    )
                    nc.sync.dma_start(out=sc[:], in_=src[r * 128 : (r + 1) * 128, :])
                    chunks.append(sc)
                out.append(chunks)
            return out

        def layer(stat, w_sb, write_out):
            # stat: list (per split) of lists (per rank) of [128, rows] tiles
            agg = []
            for c in range(nch):
                ps = agg_pool.tile([128, ic], F32, name=f"ps{c}", tag="ps")
                agg.append(ps)

            def lhs_slice(s, j):
                return stat[s][j // jt_per_rank][
                    :, (j % jt_per_rank) * 128 : (j % jt_per_rank + 1) * 128
                ]

            for g in range(ndma):
                ats = []
                for s in range(nsplit):
                    at = a_pool.tile(
                        [128, jg * rows], agg_dt, name=f"at{s}", tag=f"at{s}"
                    )
                    src = a_in[s][g * jg * 128 : (g + 1) * jg * 128, :]
                    if jg > 1:
                        nc.sync.dma_start(
                            out=at[:].rearrange("p (t i) -> p t i", t=jg),
                            in_=src.rearrange("(t p) i -> p t i", p=128),
                        )
                    else:
                        nc.sync.dma_start(out=at[:], in_=src)
                    ats.append(at)
                for t in range(jg):
                    j = g * jg + t
                    first, last = (j == 0), (j == jt - 1)
                    # pass list: (lhs split idx, rhs split idx)
                    passes = [(0, 0)]
                    if nsplit == 2:
                        passes += [(1, 0), (0, 1)]
                    for pi, (ls, rs) in enumerate(passes):
                        lhs = lhs_slice(ls, j)
                        for c in range(nch):
                            nc.tensor.matmul(
                                agg[c][:],
                                lhsT=lhs,
                                rhs=ats[rs][
                                    :, t * rows + c * ic : t * rows + (c + 1) * ic
                                ],
                                start=(first and pi == 0),
                                stop=(last and pi == len(passes) - 1),
                            )
            # linear + relu, node-major output tiles
            for c in range(nch):
                mt = m_pool.tile([128, ic], F32, name="mt", tag="mt")
                nc.vector.tensor_copy(out=mt[:], in_=agg[c][:])
                for it in range(lt):
                    lp = lin_pool.tile([128, d], F32, name="lp", tag="lp")
                    nc.tensor.matmul(
                        lp[:],
                        lhsT=mt[:, it * 128 : (it + 1) * 128],
                        rhs=w_sb[:],
                        start=True,
                        stop=True,
                    )
                    ht = h_pool.tile([128, d], F32, name="ht", tag="ht")
                    nc.scalar.activation(ht[:], lp[:], relu)
                    write_out(c * lt + it, ht)

        # ---- layer 0 ----
        stat0 = load_stat_chunks([x[:] for x in x_in])
        # packed hidden-state bounce: [hi | lo] along free dim when split
        h_tb = dram.tile([128, nsplit * rows], agg_dt, name="h_tb")

        def write_l0(tl, ht):
            if precision == "fp32":
                nc.sync.dma_start(out=h_tb[:, tl * 128 : (tl + 1) * 128], in_=ht[:])
                return
            hh = split_pool.tile([128, d], BF16, name="hh", tag="hh")
            nc.vector.tensor_copy(out=hh[:], in_=ht[:])
            nc.sync.dma_start(out=h_tb[:, tl * 128 : (tl + 1) * 128], in_=hh[:])
            if nsplit == 2:
                hh32 = split_pool.tile([128, d], F32, name="hh32", tag="hh32")
                nc.vector.tensor_copy(out=hh32[:], in_=hh[:])
                hl = split_pool.tile([128, d], BF16, name="hl", tag="hl")
                nc.vector.tensor_sub(out=hl[:], in0=ht[:], in1=hh32[:])
                nc.sync.dma_start(
                    out=h_tb[:, rows + tl * 128 : rows + (tl + 1) * 128], in_=hl[:]
                )

        layer(stat0, w0_sb, write_l0)

        # ---- exchange ----
        h_ag = dram.tile(
            [ncores * 128, nsplit * rows], agg_dt, addr_space="Shared", name="h_ag"
        )
        nc.gpsimd.collective_compute(
            "AllGather",
            mybir.AluOpType.bypass,
            replica_groups=[list(range(ncores))],
            ins=[h_tb[:]],
            outs=[h_ag[:]],
        )

        # ---- layer 1 ----
        stat1 = load_stat_chunks(
            [h_ag[:, s * rows : (s + 1) * rows] for s in range(nsplit)]
        )

        def write_l1(tl, ht):
            nc.sync.dma_start(out=h_out[tl * 128 : (tl + 1) * 128, :], in_=ht[:])

        layer(stat1, w1_sb, write_l1)

    nc.finalize()
    return nc


def _tile_stat(X, ncores, jt_per_rank):
    rows = jt_per_rank * 128
    return np.ascontiguousarray(
        X.reshape(ncores, jt_per_rank, 128, D).transpose(0, 2, 1, 3)
        .reshape(ncores * 128, rows)
    )


def shard_inputs(A_norm, X, n_nodes=N_NODES, ncores=NCORES, precision=PRECISION):
    """Host-side shard prep. Returns per-core input maps."""
    import ml_dtypes

    bf16 = ml_dtypes.bfloat16
    rows = n_nodes // ncores
    jt_per_rank = (n_nodes // 128) // ncores

    x_t = _tile_stat(X, ncores, jt_per_rank)
    if precision == "fp32":
        xs = [x_t]
    else:
        x_hi = x_t.astype(bf16)
        xs = [x_hi]
        if precision == "split3":
            xs.append((x_t - x_hi.astype(np.float32)).astype(bf16))

    in_maps = []
    for c in range(ncores):
        a_tc = np.ascontiguousarray(A_norm[c * rows : (c + 1) * rows, :].T)
        m = {}
        if precision == "fp32":
            m["a0"] = a_tc
        else:
            a_hi = a_tc.astype(bf16)
            m["a0"] = a_hi
            if precision == "split3":
                m["a1"] = (a_tc - a_hi.astype(np.float32)).astype(bf16)
        for s, x in enumerate(xs):
            m[f"x{s}"] = x
        in_maps.append(m)
    return in_maps


_CACHED = {}


def kernel(A_norm, X, W0, W1):
    A_norm = np.ascontiguousarray(A_norm, dtype=np.float32)
    X = np.ascontiguousarray(X, dtype=np.float32)
    W0 = np.ascontiguousarray(W0, dtype=np.float32)
    W1 = np.ascontiguousarray(W1, dtype=np.float32)

    from concourse.bass_utils import run_bass_kernel_spmd

    if "nc" not in _CACHED:
        _CACHED["nc"] = build_gcn()
    nc = _CACHED["nc"]

    in_maps = shard_inputs(A_norm, X)
    for m in in_maps:
        m["w0"] = W0
        m["w1"] = W1

    res = run_bass_kernel_spmd(nc, in_maps, core_ids=list(range(NCORES)))
    return np.concatenate([res.results[c]["h_out"] for c in range(NCORES)], axis=0)


# revision 5
# speedup vs baseline: 2.2446x; 2.2446x over previous
"""2-layer dense GCN on 8 Trainium2 NeuronCores.

Reference computation (all fp32):
    H0 = relu((A_norm @ X) @ W0)
    H1 = relu((A_norm @ H0) @ W1)
A_norm: [16384, 16384], X: [16384, 128], W0/W1: [128, 128].

Sharding: 1D row partition of A_norm (2048 rows/core). Each core holds
A[rows_c].T (host-transposed so the node-contraction dim lands on SBUF
partitions), computes its row block of each layer, and the hidden state
is exchanged between layers with an on-device AllGather.

Device layout is transpose-free:
  - aggregate:  psum[d, i] += X_tile[j, d].T @ A_T_tile[j, i]
                (lhsT = stationary node-major X/H tile, rhs = A^T slice)
  - linear:     psum[i, e]  = M^T_tile[d, i].T @ W[d, e]   (node-major out)
  - relu fused into the PSUM->SBUF eviction on the scalar engine.

The stationary H layout in SBUF ([128, rows] chunks, one per rank) is
exactly what AllGather produces, so no transposes are needed anywhere;
the only data reshuffle is the host-side A[rows].T at sharding time.

PRECISION modes:
  - "fp32":   exact fp32 matmuls (4 cyc/row on the PE).
  - "split3": A and X/H split into bf16 hi+lo; aggregate computed as
              Ah@Xh + Ah@Xl + Al@Xh (3 bf16 passes, 3 cyc/row,
              ~6e-6 relative error — fp32-class).
  - "bf16":   plain bf16 aggregate (1 cyc/row, half the DMA bytes,
              ~3e-3 relative error).
"""

import sys
from contextlib import ExitStack

if "/opt/trn_rl_repo" not in sys.path:
    sys.path.insert(0, "/opt/trn_rl_repo")

import numpy as np

N_NODES = 16384
D = 128
NCORES = 8
ROWS = N_NODES // NCORES  # 2048

PRECISION = "fp32"  # "fp32" | "split3" | "bf16"


def build_gcn(n_nodes=N_NODES, d=D, ncores=NCORES, precision=PRECISION):
    """Build the SPMD Bass program (one program, runs on all cores)."""
    import concourse.bass as bass  # noqa: F401
    import concourse.tile as tile
    from concourse import bacc, mybir

    F32 = mybir.dt.float32
    BF16 = mybir.dt.bfloat16
    agg_dt = F32 if precision == "fp32" else BF16
    esz = 4 if precision == "fp32" else 2
    nsplit = 2 if precision == "split3" else 1  # hi/lo operand copies

    rows = n_nodes // ncores
    jt = n_nodes // 128          # total j-tiles (contraction tiles)
    jt_per_rank = jt // ncores   # j-tiles covered by one stat chunk
    ic = min(512, rows)          # i-chunk width (one PSUM bank, fp32 out)
    nch = rows // ic             # i-chunks per core
    lt = ic // 128               # linear i-tiles per chunk
    # j-tiles per A DMA: target ~2 MiB per transfer
    jg = max(1, (2 * 1024 * 1024) // (128 * rows * esz))
    jg = min(jg, jt_per_rank)
    ndma = jt // jg

    nc = bacc.Bacc("TRN2", target_bir_lowering=False, num_devices=ncores)

    # A^T shards (hi always; lo only for split3)
    a_in = [
        nc.dram_tensor(f"a{s}", [n_nodes, rows], agg_dt, kind="ExternalInput")
        for s in range(nsplit)
    ]
    # x_t: X pre-tiled on host into the same layout AllGather produces:
    # x_t[r*128 + p, tl*128 + dd] = X[(r*jt_per_rank + tl)*128 + p, dd]
    x_in = [
        nc.dram_tensor(f"x{s}", [ncores * 128, rows], agg_dt, kind="ExternalInput")
        for s in range(nsplit)
    ]
    w0 = nc.dram_tensor("w0", [d, d], F32, kind="ExternalInput")
    w1 = nc.dram_tensor("w1", [d, d], F32, kind="ExternalInput")
    h_out = nc.dram_tensor("h_out", [rows, d], F32, kind="ExternalOutput")

    relu = mybir.ActivationFunctionType.Relu

    with tile.TileContext(nc) as tc, ExitStack() as ctx:
        sb1 = ctx.enter_context(tc.tile_pool(name="sb1", bufs=1))
        stat_pool = ctx.enter_context(
            tc.tile_pool(name="stat", bufs=ncores * nsplit)
        )
        a_pool = ctx.enter_context(tc.tile_pool(name="a", bufs=4 * nsplit))
        m_pool = ctx.enter_context(tc.tile_pool(name="m", bufs=2))
        h_pool = ctx.enter_context(tc.tile_pool(name="h", bufs=4))
        split_pool = ctx.enter_context(tc.tile_pool(name="spl", bufs=4))
        agg_pool = ctx.enter_context(tc.tile_pool(name="agg", bufs=nch, space="PSUM"))
        lin_pool = ctx.enter_context(tc.tile_pool(name="lin", bufs=2, space="PSUM"))
        dram = ctx.enter_context(tc.tile_pool(name="dram", bufs=1, space="DRAM"))

        w0_sb = sb1.tile([d, d], F32)
        nc.sync.dma_start(out=w0_sb[:], in_=w0[:])
        w1_sb = sb1.tile([d, d], F32)
        nc.sync.dma_start(out=w1_sb[:], in_=w1[:])

        def load_stat_chunks(srcs):
            """srcs: list of [ncores*128, rows] DRAM views -> per-split chunk
            tile lists."""
            out = []
            for s, src in enumerate(srcs):
                chunks = []
                for r in range(ncores):
                    sc = stat_pool.tile(
                        [128, rows], agg_dt, name=f"sc{s}_{r}", tag="sc"
                    )
                    nc.sync.dma_start(out=sc[:], in_=src[r * 128 : (r + 1) * 128, :])
                    chunks.append(sc)
                out.append(chunks)
            return out

        def layer(stat, w_sb, write_out):
            # stat: list (per split) of lists (per rank) of [128, rows] tiles
            agg = []
            for c in range(nch):
                ps = agg_pool.tile([128, ic], F32, name=f"ps{c}", tag="ps")
                agg.append(ps)

            def lhs_slice(s, j):
                return stat[s][j // jt_per_rank][
                    :, (j % jt_per_rank) * 128 : (j % jt_per_rank + 1) * 128
                ]

            for g in range(ndma):
                ats = []
                for s in range(nsplit):
                    at = a_pool.tile(
                        [128, jg * rows], agg_dt, name=f"at{s}", tag=f"at{s}"
                    )
                    src = a_in[s][g * jg * 128 : (g + 1) * jg * 128, :]
                    if jg > 1:
                        nc.sync.dma_start(
                            out=at[:].rearrange("p (t i) -> p t i", t=jg),
                            in_=src.rearrange("(t p) i -> p t i", p=128),
                        )
                    else:
                        nc.sync.dma_start(out=at[:], in_=src)
                    ats.append(at)
                for t in range(jg):
                    j = g * jg + t
                    first, last = (j == 0), (j == jt - 1)
                    # pass list: (lhs split idx, rhs split idx)
                    passes = [(0, 0)]
                    if nsplit == 2:
                        passes += [(1, 0), (0, 1)]
                    for pi, (ls, rs) in enumerate(passes):
                        lhs = lhs_slice(ls, j)
                        for c in range(nch):
                            nc.tensor.matmul(
                                agg[c][:],
                                lhsT=lhs,
                                rhs=ats[rs][
                                    :, t * rows + c * ic : t * rows + (c + 1) * ic
                                ],
                                start=(first and pi == 0),
                                stop=(last and pi == len(passes) - 1),
                            )
            # linear + relu, node-major output tiles
            for c in range(nch):
                mt = m_pool.tile([128, ic], F32, name="mt", tag="mt")
                nc.vector.tensor_copy(out=mt[:], in_=agg[c][:])
                for it in range(lt):
                    lp = lin_pool.tile([128, d], F32, name="lp", tag="lp")
                    nc.tensor.matmul(
                        lp[:],
                        lhsT=mt[:, it * 128 : (it + 1) * 128],
                        rhs=w_sb[:],
                        start=True,
                        stop=True,
                    )
                    ht = h_pool.tile([128, d], F32, name="ht", tag="ht")
                    nc.scalar.activation(ht[:], lp[:], relu)
                    write_out(c * lt + it, ht)

        # ---- layer 0 ----
        stat0 = load_stat_chunks([x[:] for x in x_in])
        # packed hidden-state bounce: [hi | lo] along free dim when split
        h_tb = dram.tile([128, nsplit * rows], agg_dt, name="h_tb")

        def write_l0(tl, ht):
            if precision == "fp32":
                nc.sync.dma_start(out=h_tb[:, tl * 128 : (tl + 1) * 128], in_=ht[:])
                return
            hh = split_pool.tile([128, d], BF16, name="hh", tag="hh")
            nc.vector.tensor_copy(out=hh[:], in_=ht[:])
            nc.sync.dma_start(out=h_tb[:, tl * 128 : (tl + 1) * 128], in_=hh[:])
            if nsplit == 2:
                hh32 = split_pool.tile([128, d], F32, name="hh32", tag="hh32")
                nc.vector.tensor_copy(out=hh32[:], in_=hh[:])
                hl = split_pool.tile([128, d], BF16, name="hl", tag="hl")
                nc.vector.tensor_sub(out=hl[:], in0=ht[:], in1=hh32[:])
                nc.sync.dma_start(
                    out=h_tb[:, rows + tl * 128 : rows + (tl + 1) * 128], in_=hl[:]
                )

        layer(stat0, w0_sb, write_l0)

        # ---- exchange ----
        h_ag = dram.tile(
            [ncores * 128, nsplit * rows], agg_dt, addr_space="Shared", name="h_ag"
        )
        nc.gpsimd.collective_compute(
            "AllGather",
            mybir.AluOpType.bypass,
            replica_groups=[list(range(ncores))],
            ins=[h_tb[:]],
            outs=[h_ag[:]],
        )

        # ---- layer 1 ----
        stat1 = load_stat_chunks(
            [h_ag[:, s * rows : (s + 1) * rows] for s in range(nsplit)]
        )

        def write_l1(tl, ht):
            nc.sync.dma_start(out=h_out[tl * 128 : (tl + 1) * 128, :], in_=ht[:])

        layer(stat1, w1_sb, write_l1)

    nc.finalize()
    return nc


def _tile_stat(X, ncores, jt_per_rank):
    rows = jt_per_rank * 128
    return np.ascontiguousarray(
        X.reshape(ncores, jt_per_rank, 128, D).transpose(0, 2, 1, 3)
        .reshape(ncores * 128, rows)
    )


def shard_inputs(A_norm, X, n_nodes=N_NODES, ncores=NCORES, precision=PRECISION):
    """Host-side shard prep. Returns per-core input maps."""
    import ml_dtypes

    bf16 = ml_dtypes.bfloat16
    rows = n_nodes // ncores
    jt_per_rank = (n_nodes // 128) // ncores

    x_t = _tile_stat(X, ncores, jt_per_rank)
    if precision == "fp32":
        xs = [x_t]
    else:
        x_hi = x_t.astype(bf16)
        xs = [x_hi]
        if precision == "split3":
            xs.append((x_t - x_hi.astype(np.float32)).astype(bf16))

    in_maps = []
    for c in range(ncores):
        a_tc = np.ascontiguousarray(A_norm[c * rows : (c + 1) * rows, :].T)
        m = {}
        if precision == "fp32":
            m["a0"] = a_tc
        else:
            a_hi = a_tc.astype(bf16)
            m["a0"] = a_hi
            if precision == "split3":
                m["a1"] = (a_tc - a_hi.astype(np.float32)).astype(bf16)
        for s, x in enumerate(xs):
            m[f"x{s}"] = x
        in_maps.append(m)
    return in_maps


_CACHED = {}


def kernel(A_norm, X, W0, W1):
    A_norm = np.ascontiguousarray(A_norm, dtype=np.float32)
    X = np.ascontiguousarray(X, dtype=np.float32)
    W0 = np.ascontiguousarray(W0, dtype=np.float32)
    W1 = np.ascontiguousarray(W1, dtype=np.float32)

    from concourse.bass_utils import run_bass_kernel_spmd

    if PRECISION not in _CACHED:
        _CACHED[PRECISION] = build_gcn(precision=PRECISION)
    nc = _CACHED[PRECISION]

    in_maps = shard_inputs(A_norm, X, precision=PRECISION)
    for m in in_maps:
        m["w0"] = W0
        m["w1"] = W1

    res = run_bass_kernel_spmd(nc, in_maps, core_ids=list(range(NCORES)))
    return np.concatenate([res.results[c]["h_out"] for c in range(NCORES)], axis=0)
